# revision 14
# baseline (speedup 1.0000x reference)
"""MaxUnpooling2D scatter-add kernel for Trainium2 (8 NeuronCores).

Reference semantics (per batch b):
    y = mask // (OW*C); x = (mask // C) % OW; f = channel index c
    out[b, y, x, c] += updates[b, h, w, c]      (duplicates sum)

Strategy (pure data-parallel over batch; 2 batches per core):
  - SBUF input layout [128 partitions, 4096]: partition p holds hw rows
    [32p, 32p+32), free column j = q*128 + c  (q in [0,32), c = channel).
  - Per (plane c, chunk q): 128 elements (one per partition) scatter-routed
    with a dense one-hot matmul:
        A[i, y]  = (iota == Y[i])             (stationary)
        Bv[i, x] = (iota == X[i]) * V[i]      (moving)
        psum_c[y, x] += A.T @ Bv              (PE contraction over i)
    PSUM f32 accumulates the 32 chunks of a plane; duplicates sum exactly.

V6 (the kernel is DVE-bound; spread the V-mult to idle engines without the
V4 pipeline-stall failure mode):
  - A one-hot always in the q-interleaved layout [p, qh=16, w=128, ql=2]
    (chunk q = 2qh+ql; DVE-2x build, 4-byte-stride LDWEIGHTS slices).
  - V-mult engine by plane: c%4==0 -> ACT (32 chunk ops, per-partition
    scale from the f32 input; contiguous movings), c%4==2 -> Pool (batched
    gpsimd mult on interleaved layout), odd c -> DVE (plain-layout batched
    mult — the plain 3D-ish APs keep the mult at the full 2x rate, the
    64B-stride movings only cost PE time which has slack).
  - Plane PAIRS (even, odd) are processed with chunk-interleaved matmul
    chains: every pair couples a streamed/offloaded plane with a cheap DVE
    plane, so the chain is never gated by a monolithic slow producer, and
    consecutive PE matmuls alternate PSUM banks.
  - PSUM evac on ACT, deferred one pair for pipelining.
"""

import sys

sys.path.insert(0, "/opt/trn_rl_repo")

import numpy as np

import concourse.bacc as bacc
import concourse.bass as bass
import concourse.tile as tile
from concourse import mybir
from concourse.bass_utils import run_bass_kernel_spmd

# Problem shape (hardcoded per contract)
B, H, W, C = 16, 64, 64, 128
OH, OW = 2 * H, 2 * W
N_CORES = 8
B_PER_CORE = B // N_CORES  # 2
HWF = H * W  # 4096
P = 128
Q = HWF // P  # 32 hw rows per partition
Q2 = Q // 2  # 16 interleaved chunk pairs
NCOL = Q * C  # 4096

F32 = mybir.dt.float32
BF16 = mybir.dt.bfloat16
FP16 = mybir.dt.float16
I32 = mybir.dt.int32


def _mult_engine(c):
    if c % 4 == 0:
        return "act"
    if c % 4 == 2:
        return "pool"
    return "dve"


def build_nc(n_planes=C, dt=FP16):
    nc = bacc.Bacc("TRN2", target_bir_lowering=False, debug=False)

    upd = nc.declare_dram_parameter("updates", [B_PER_CORE, HWF, C], F32, isOutput=False)
    msk = nc.declare_dram_parameter("mask", [B_PER_CORE, HWF, C], I32, isOutput=False)
    iota_in = nc.declare_dram_parameter("iota", [P, P], F32, isOutput=False)
    out = nc.declare_dram_parameter("out", [B_PER_CORE, OH, OW, C], F32, isOutput=True)

    with tile.TileContext(nc) as tc:
        with (
            tc.tile_pool(name="const", bufs=1) as const_pool,
            tc.tile_pool(name="inp", bufs=1) as inp_pool,
            tc.tile_pool(name="pl", bufs=1) as pl_pool,
            tc.tile_pool(name="apool", bufs=3) as a_pool,
            tc.tile_pool(name="xpool", bufs=2) as x_pool,
            tc.tile_pool(name="bpool", bufs=3) as b_pool,
            tc.tile_pool(name="bch", bufs=8) as bch_pool,
            tc.tile_pool(name="psum", bufs=8, space="PSUM") as psum_pool,
        ):
            iota_f = const_pool.tile([P, P], F32)
            nc.sync.dma_start(iota_f[:], iota_in[:])
            # iota2[p, w, ql] = w (fp16): serves both build layouts with an
            # innermost stride-1 pair so every build op stays in DVE 2x mode
            iota2 = const_pool.tile([P, P, 2], dt)
            nc.vector.tensor_copy(
                iota2[:],
                iota_f[:]
                .rearrange("p (w o) -> p w o", o=1)
                .broadcast_to([P, P, 2]),
            )

            # interleaved layout [p, qh, w, ql] operand views
            def il_iota():
                return (
                    iota2[:]
                    .rearrange("p (o w) ql -> p o w ql", o=1)
                    .broadcast_to([P, Q2, P, 2])
                )

            def il_col(tile_, c):
                return (
                    tile_[:, c, :]
                    .rearrange("p (qh o ql) -> p qh o ql", o=1, ql=2)
                    .broadcast_to([P, Q2, P, 2])
                )

            # plain layout [p, w, q] operand views (as [p, w, qh, ql])
            def pl_iota():
                return (
                    iota2[:]
                    .rearrange("p w (o ql) -> p w o ql", o=1)
                    .broadcast_to([P, P, Q2, 2])
                )

            def pl_col(tile_, c):
                return (
                    tile_[:, c, :]
                    .rearrange("p (qh o ql) -> p o qh ql", o=1, ql=2)
                    .broadcast_to([P, P, Q2, 2])
                )

            for b in range(B_PER_CORE):
                # ---- load batch b ----
                u_f = inp_pool.tile([P, NCOL], F32, tag="uf")
                nc.sync.dma_start(u_f[:], upd[b].rearrange("(p q) c -> p (q c)", p=P))
                m = inp_pool.tile([P, NCOL], I32, tag="m")
                nc.sync.dma_start(m[:], msk[b].rearrange("(p q) c -> p (q c)", p=P))

                # ---- decode mask -> channel-major fp16 Y/X/V tiles [p, c, q] ----
                yi = inp_pool.tile([P, NCOL], I32, tag="yi")
                nc.vector.tensor_scalar(
                    yi[:], m[:], 14, None, mybir.AluOpType.logical_shift_right
                )
                ytr = inp_pool.tile([P, C, Q], dt, tag="ytr")
                nc.vector.tensor_copy(ytr[:], yi[:].rearrange("p (q c) -> p c q", c=C))

                xi = inp_pool.tile([P, NCOL], I32, tag="yi")
                nc.vector.tensor_scalar(
                    xi[:],
                    m[:],
                    7,
                    127,
                    mybir.AluOpType.logical_shift_right,
                    mybir.AluOpType.bitwise_and,
                )
                xtr = inp_pool.tile([P, C, Q], dt, tag="xtr")
                nc.vector.tensor_copy(xtr[:], xi[:].rearrange("p (q c) -> p c q", c=C))

                vtr = inp_pool.tile([P, C, Q], dt, tag="vtr")
                nc.vector.tensor_copy(vtr[:], u_f[:].rearrange("p (q c) -> p c q", c=C))

                pl = pl_pool.tile([P, P, C], F32)  # [y, x, c]
                if n_planes < C:
                    nc.gpsimd.memset(pl[:], 0.0)

                pending_evac = None
                for cp in range(0, n_planes, 2):
                    pair = [cp, cp + 1] if cp + 1 < n_planes else [cp]
                    movings, accs = [], []
                    for c in pair:
                        eng = _mult_engine(c)
                        # A build: interleaved layout, DVE 2x
                        a2 = a_pool.tile([P, Q2, P, 2], dt, tag="a")
                        nc.vector.tensor_tensor(
                            a2[:], il_iota(), il_col(ytr, c), mybir.AluOpType.is_equal
                        )

                        if eng == "dve":
                            # plain-layout X-eq + V-mult (full-rate DVE 2x)
                            xeq = x_pool.tile([P, P, Q], dt, tag="xeq")
                            xeq4 = xeq[:].rearrange("p w (qh ql) -> p w qh ql", ql=2)
                            nc.vector.tensor_tensor(
                                xeq4, pl_iota(), pl_col(xtr, c), mybir.AluOpType.is_equal
                            )
                            b2 = b_pool.tile([P, P, Q], dt, tag="b")
                            nc.vector.tensor_tensor(
                                b2[:].rearrange("p w (qh ql) -> p w qh ql", ql=2),
                                xeq4,
                                pl_col(vtr, c),
                                mybir.AluOpType.mult,
                            )
                            movings.append(lambda qh, ql, t=b2: t[:, :, 2 * qh + ql])
                        elif eng == "pool":
                            # interleaved X-eq (DVE) + batched Pool mult
                            xeq = x_pool.tile([P, Q2, P, 2], dt, tag="xeq")
                            nc.vector.tensor_tensor(
                                xeq[:], il_iota(), il_col(xtr, c),
                                mybir.AluOpType.is_equal,
                            )
                            b2 = b_pool.tile([P, Q2, P, 2], dt, tag="b")
                            nc.gpsimd.tensor_tensor(
                                b2[:], xeq[:], il_col(vtr, c), mybir.AluOpType.mult
                            )
                            movings.append(lambda qh, ql, t=b2: t[:, qh, :, ql])
                        else:  # act
                            # interleaved X-eq (DVE) + 32 ACT chunk mults
                            xeq = x_pool.tile([P, Q2, P, 2], dt, tag="xeq")
                            nc.vector.tensor_tensor(
                                xeq[:], il_iota(), il_col(xtr, c),
                                mybir.AluOpType.is_equal,
                            )
                            chunks = []
                            for q in range(Q):
                                qh, ql = q // 2, q % 2
                                b_ch = bch_pool.tile([P, P], dt, tag="bch")
                                nc.scalar.activation(
                                    b_ch[:],
                                    xeq[:, qh, :, ql],
                                    mybir.ActivationFunctionType.Copy,
                                    bias=0.0,
                                    scale=u_f[:, q * C + c : q * C + c + 1],
                                )
                                chunks.append(b_ch)
                            movings.append(
                                lambda qh, ql, ch=chunks: ch[2 * qh + ql][:]
                            )
                        acc = psum_pool.tile([P, P], F32, tag="acc")  # [y, x]
                        accs.append((acc, a2))

                    # interleaved accumulation chains across the pair's banks
                    for qh in range(Q2):
                        for ql in range(2):
                            for k in range(len(pair)):
                                acc, a2 = accs[k]
                                nc.tensor.matmul(
                                    acc[:],
                                    a2[:, qh, :, ql],
                                    movings[k](qh, ql),
                                    start=(qh == 0 and ql == 0),
                                    stop=(qh == Q2 - 1 and ql == 1),
                                )

                    # deferred evac keeps ACT a pair behind (pipelining)
                    if pending_evac is not None:
                        for acc_, c_ in pending_evac:
                            nc.scalar.copy(pl[:, :, c_], acc_[:])
                    pending_evac = [(accs[k][0], pair[k]) for k in range(len(pair))]

                if pending_evac is not None:
                    for acc_, c_ in pending_evac:
                        nc.scalar.copy(pl[:, :, c_], acc_[:])

                nc.sync.dma_start(out[b].rearrange("y x c -> y (x c)"), pl[:])

    nc.compile()
    return nc


_CACHED = {}


def _get_nc(n_planes=C):
    key = n_planes
    if key not in _CACHED:
        _CACHED[key] = build_nc(n_planes)
    return _CACHED[key]


def kernel(updates: np.ndarray, mask: np.ndarray) -> np.ndarray:
    nc = _get_nc()
    iota = np.broadcast_to(np.arange(P, dtype=np.float32), (P, P)).copy()
    in_maps = []
    for i in range(N_CORES):
        sl = slice(i * B_PER_CORE, (i + 1) * B_PER_CORE)
        in_maps.append(
            {
                "updates": np.ascontiguousarray(
                    updates[sl].reshape(B_PER_CORE, HWF, C), dtype=np.float32
                ),
                "mask": np.ascontiguousarray(
                    mask[sl].reshape(B_PER_CORE, HWF, C), dtype=np.int32
                ),
                "iota": iota,
            }
        )
    res = run_bass_kernel_spmd(nc, in_maps, list(range(N_CORES)))
    return np.concatenate([res.results[i]["out"] for i in range(N_CORES)], axis=0)


# revision 15
# speedup vs baseline: 1.1751x; 1.1751x over previous
"""MaxUnpooling2D scatter-add kernel for Trainium2 (8 NeuronCores).

Reference semantics (per batch b):
    y = mask // (OW*C); x = (mask // C) % OW; f = channel index c
    out[b, y, x, c] += updates[b, h, w, c]      (duplicates sum)

Strategy (pure data-parallel over batch; 2 batches per core):
  - Layout SBUF tiles [128 partitions, 4096] where partition p holds hw rows
    [32p, 32p+32) and free column j = q*128 + c  (q in [0,32), c = channel).
  - For each (plane c, chunk q): the 128 elements (one per partition) are
    scatter-routed with a dense one-hot matmul:
        A[i, y]  = (iota == Y[i])             (stationary operand)
        Bv[i, x] = (iota == X[i]) * V[i]      (moving operand)
        psum_c[y, x] += A.T @ Bv              (PE contraction over i)
    PSUM (f32) accumulates the 32 chunks of a plane; duplicates sum exactly.
  - Evacuate psum_c[y, x] into PL[y, x, c]; one contiguous 8MB DMA per batch.

Engine split: all one-hot builds are plane-batched DVE tensor_tensor ops in
a transposed [partition, onehot, q] layout with a MATERIALIZED iota tensor,
so every operand has an innermost step of 1 in fp16 and the ops run in the
DVE 2x packed mode (broadcasts ride on middle dims only). The matmul takes
strided [p, :, q] slices (strided LDWEIGHTS costs ~40 ns, acceptable).
PSUM evac on ACT. fp16 one-hots/values (lane ids exact; values rounded to
11 bits -> ~2e-4 rel err); PSUM accumulates in f32. Measured 1.97 ms HW
exec (DVE ~92% busy at the batched-2x floor); pool depths (apool=3,
xpool=2, bpool=2, psum=8) are load-bearing for PE/DVE overlap.

Optimization post-mortem (this session, all measured on HW):
  - q-interleaved one-hot layouts (4B-stride matmul operands) cut the
    per-matmul duration 410->308ns and pair-interleaved PSUM chains cut it
    further to ~264ns, but PE "busy" here is pipelined-duration inflation:
    the kernel's true critical path is the DVE, so these were neutral.
  - Offloading the V-mult to Pool (batched gpsimd mult, 7-13us/plane) or
    ACT (32 chunked scale-mults, ~12-15us/plane) relieves DVE arithmetic
    but introduces cross-engine pipeline stalls (in-order engine queues +
    shallow tile-pool lookahead): V4 2361us, V5 2080us, V6 2320us — all
    WORSE than this kernel's 1971us. The balanced DVE/PE co-bottleneck
    here is within ~12% of the pure-DVE floor (1.76ms busy).
"""

import sys

sys.path.insert(0, "/opt/trn_rl_repo")

import numpy as np

import concourse.bacc as bacc
import concourse.bass as bass
import concourse.tile as tile
from concourse import mybir
from concourse.bass_utils import run_bass_kernel_spmd

# Problem shape (hardcoded per contract)
B, H, W, C = 16, 64, 64, 128
OH, OW = 2 * H, 2 * W
N_CORES = 8
B_PER_CORE = B // N_CORES  # 2
HWF = H * W  # 4096
P = 128
Q = HWF // P  # 32 hw rows per partition
NCOL = Q * C  # 4096

F32 = mybir.dt.float32
BF16 = mybir.dt.bfloat16
FP16 = mybir.dt.float16
I32 = mybir.dt.int32

def build_nc(n_planes=C, repeat=1, dt=FP16):
    nc = bacc.Bacc("TRN2", target_bir_lowering=False, debug=False)

    upd = nc.declare_dram_parameter("updates", [B_PER_CORE, HWF, C], F32, isOutput=False)
    msk = nc.declare_dram_parameter("mask", [B_PER_CORE, HWF, C], I32, isOutput=False)
    iota_in = nc.declare_dram_parameter("iota", [P, P], F32, isOutput=False)
    out = nc.declare_dram_parameter("out", [B_PER_CORE, OH, OW, C], F32, isOutput=True)

    with tile.TileContext(nc) as tc:
        with (
            tc.tile_pool(name="const", bufs=1) as const_pool,
            tc.tile_pool(name="inp", bufs=1) as inp_pool,
            tc.tile_pool(name="pl", bufs=1) as pl_pool,
            tc.tile_pool(name="apool", bufs=3) as a_pool,
            tc.tile_pool(name="xpool", bufs=2) as x_pool,
            tc.tile_pool(name="bpool", bufs=2) as b_pool,
            tc.tile_pool(name="psum", bufs=8, space="PSUM") as psum_pool,
        ):
            iota_f = const_pool.tile([P, P], F32)
            nc.sync.dma_start(iota_f[:], iota_in[:])
            # materialized iotaT[p, y, q] = y  (fp16, innermost step 1) so the
            # plane-batched build ops qualify for the DVE 2x packed mode
            iotaT = const_pool.tile([P, P, Q], dt)
            nc.vector.tensor_copy(
                iotaT[:],
                iota_f[:]
                .rearrange("p (y o) -> p y o", o=1)
                .broadcast_to([P, P, Q]),
            )

            for b_rep in range(B_PER_CORE * repeat):
                b = b_rep % B_PER_CORE
                # ---- load batch b ----
                u_f = inp_pool.tile([P, NCOL], F32, tag="uf")
                nc.sync.dma_start(u_f[:], upd[b].rearrange("(p q) c -> p (q c)", p=P))
                m = inp_pool.tile([P, NCOL], I32, tag="m")
                nc.sync.dma_start(m[:], msk[b].rearrange("(p q) c -> p (q c)", p=P))

                # ---- decode mask -> channel-major fp16 Y/X/V tiles [p, c, q] ----
                yi = inp_pool.tile([P, NCOL], I32, tag="yi")
                nc.vector.tensor_scalar(
                    yi[:], m[:], 14, None, mybir.AluOpType.logical_shift_right
                )
                ytr = inp_pool.tile([P, C, Q], dt, tag="ytr")
                nc.vector.tensor_copy(ytr[:], yi[:].rearrange("p (q c) -> p c q", c=C))

                xi = inp_pool.tile([P, NCOL], I32, tag="yi")
                nc.vector.tensor_scalar(
                    xi[:],
                    m[:],
                    7,
                    127,
                    mybir.AluOpType.logical_shift_right,
                    mybir.AluOpType.bitwise_and,
                )
                xtr = inp_pool.tile([P, C, Q], dt, tag="xtr")
                nc.vector.tensor_copy(xtr[:], xi[:].rearrange("p (q c) -> p c q", c=C))

                vtr = inp_pool.tile([P, C, Q], dt, tag="vtr")
                nc.vector.tensor_copy(vtr[:], u_f[:].rearrange("p (q c) -> p c q", c=C))

                pl = pl_pool.tile([P, P, C], F32)  # [y, x, c]
                if n_planes < C:
                    nc.gpsimd.memset(pl[:], 0.0)

                for c in range(n_planes):
                    # plane-batched builds, all at DVE 2x (fp16, step-1 inner):
                    # a[p, y, q] = (iotaT == Ytr[p,c,q]); b = (iotaT == X) * V
                    a_pl = a_pool.tile([P, P, Q], dt, tag="a")
                    y_bc = (
                        ytr[:, c, :]
                        .rearrange("p (o q) -> p o q", o=1)
                        .broadcast_to([P, P, Q])
                    )
                    nc.vector.tensor_tensor(
                        a_pl[:], iotaT[:], y_bc, mybir.AluOpType.is_equal
                    )
                    xeq = x_pool.tile([P, P, Q], dt, tag="xeq")
                    x_bc = (
                        xtr[:, c, :]
                        .rearrange("p (o q) -> p o q", o=1)
                        .broadcast_to([P, P, Q])
                    )
                    nc.vector.tensor_tensor(
                        xeq[:], iotaT[:], x_bc, mybir.AluOpType.is_equal
                    )
                    b_pl = b_pool.tile([P, P, Q], dt, tag="b")
                    v_bc = (
                        vtr[:, c, :]
                        .rearrange("p (o q) -> p o q", o=1)
                        .broadcast_to([P, P, Q])
                    )
                    nc.vector.tensor_tensor(
                        b_pl[:], xeq[:], v_bc, mybir.AluOpType.mult
                    )

                    acc = psum_pool.tile([P, P], F32)  # [y, x]
                    for q in range(Q):
                        # psum[y, x] += sum_i a[i, y] * b[i, x]
                        nc.tensor.matmul(
                            acc[:],
                            a_pl[:, :, q],
                            b_pl[:, :, q],
                            start=(q == 0),
                            stop=(q == Q - 1),
                        )
                    # evacuate plane: pl[:, :, c] = acc
                    nc.scalar.copy(pl[:, :, c], acc[:])

                nc.sync.dma_start(out[b].rearrange("y x c -> y (x c)"), pl[:])

    nc.compile()
    return nc


_CACHED = {}


def _get_nc(n_planes=C):
    key = n_planes
    if key not in _CACHED:
        _CACHED[key] = build_nc(n_planes)
    return _CACHED[key]


def kernel(updates: np.ndarray, mask: np.ndarray) -> np.ndarray:
    nc = _get_nc()
    iota = np.broadcast_to(np.arange(P, dtype=np.float32), (P, P)).copy()
    in_maps = []
    for i in range(N_CORES):
        sl = slice(i * B_PER_CORE, (i + 1) * B_PER_CORE)
        in_maps.append(
            {
                "updates": np.ascontiguousarray(
                    updates[sl].reshape(B_PER_CORE, HWF, C), dtype=np.float32
                ),
                "mask": np.ascontiguousarray(
                    mask[sl].reshape(B_PER_CORE, HWF, C), dtype=np.int32
                ),
                "iota": iota,
            }
        )
    res = run_bass_kernel_spmd(nc, in_maps, list(range(N_CORES)))
    return np.concatenate([res.results[i]["out"] for i in range(N_CORES)], axis=0)
